# revision 38
# baseline (speedup 1.0000x reference)
"""DeepSeek-V3 MoE routing kernel for 8x Trainium2 NeuronCores.

Strategy (expert-parallel, dense-per-core):
- 256 experts sharded 32/core. Gate (sigmoid + grouped top-k routing) is
  replicated on every core; per-core inputs are group-rotated so each core's
  32 local experts always occupy combine columns 0..31 (SPMD-friendly).
- Each core computes all 256 tokens through its 32 experts (dense), scales
  the intermediate activations by the combine weights, and accumulates the
  down-projections of all its experts (plus a 32-wide slice of the shared
  expert) directly in PSUM. Partial outputs are summed with an AllReduce.
- Expert matmuls run in float32r (reduced-precision fp32, full PE rate);
  the gate matmul runs in full fp32 so top-k decisions match the reference.
- Expert weights stream from HBM in 2-expert (2 MB) SWDGE DMAs that cast
  f32 -> f32r in flight; this streaming is the bottleneck resource.

PSUM budget (8 banks): Y accumulator 4 + h1h3 double-buffer 2 + routing 2.
"""
import numpy as np

from concourse import bacc, tile
import concourse.mybir as mybir
from concourse.bass_utils import run_bass_kernel_spmd

E = 256
H = 1024
I = 256
T = 256
N_GROUP = 8
TOPK_GROUP = 4
TOP_K = 8
SCALE = 2.5
N_CORES = 8
EL = E // N_CORES          # local experts per core (= one routing group)
IS = I // N_CORES          # shared-expert intermediate slice per core
HC = H // 128              # h chunks
TC = T // 128              # token chunks
IC = I // 128              # intermediate chunks

fp32 = mybir.dt.float32
fp32r = mybir.dt.float32r
i32 = mybir.dt.int32
Alu = mybir.AluOpType
Act = mybir.ActivationFunctionType

_NC_CACHE = {}


def build_nc(single_core=False, w_bufs=4, ahead=4, pre_n=4):
    # w_bufs applies to both the up (16KB) and wd (8KB) tags
    nc = bacc.Bacc("TRN2", debug=False, num_devices=1 if single_core else N_CORES)

    # host passes pre-swizzled layouts (pure layout transforms, no compute):
    #  xt   [128, HC, T]   : xt[p, c, t] = x[t, c*128+p]
    #  gwt  [128, HC, E]   : gwt[p, c, e] = gate_w_perm[e, c*128+p]
    #  wblob[EL, 128, 3, 2048]: per expert, partition-major pack of
    #       w1 (hc, i), w3 (hc, i), wd (ic, h)
    #  swgt/swut [128, HC, IS]; swd [IS, H]
    XT = nc.dram_tensor("xt", [128, HC, T], fp32, kind="ExternalInput")
    GWT = nc.dram_tensor("gwt", [128, HC, E], fp32, kind="ExternalInput")
    EB = nc.dram_tensor("ebp", [E], fp32, kind="ExternalInput")
    WBU = nc.dram_tensor("wbu", [EL, 128, 2, 2048], fp32, kind="ExternalInput")
    WBD = nc.dram_tensor("wbd", [EL, 128, 2048], fp32, kind="ExternalInput")
    SWGT = nc.dram_tensor("swgt", [128, HC, IS], fp32, kind="ExternalInput")
    SWUT = nc.dram_tensor("swut", [128, HC, IS], fp32, kind="ExternalInput")
    SWD = nc.dram_tensor("swd", [IS, H], fp32, kind="ExternalInput")
    Y = nc.dram_tensor("y", [T, H], fp32, kind="ExternalOutput")

    with tile.TileContext(nc) as tc:
        with (
            tc.tile_pool(name="persist", bufs=1) as pp,
            tc.tile_pool(name="route", bufs=1) as rp,
            tc.tile_pool(name="wpool", bufs=w_bufs) as wp,
            tc.tile_pool(name="spool", bufs=2) as sp,
            tc.tile_pool(name="s1pool", bufs=1) as s1p,
            tc.tile_pool(name="a13pool", bufs=5) as a13p,
            tc.tile_pool(name="hpsum", bufs=2, space="PSUM") as hp,
            tc.tile_pool(name="dram", bufs=1, space="DRAM") as dp,
        ):
          with tc.tile_pool(name="tpsum", bufs=3, space="PSUM") as tp:
            # tiny identity (for the combine transpose) built on DVE/Pool
            colI = rp.tile([128, 1], i32)
            nc.gpsimd.iota(colI[:], [[0, 1]], channel_multiplier=1, base=0)
            colF = rp.tile([128, 1], fp32)
            nc.vector.tensor_copy(colF[:], colI[:])
            rowI = rp.tile([128, 128], i32)
            nc.gpsimd.iota(rowI[:], [[1, 128]], channel_multiplier=0, base=0)
            rowF = rp.tile([128, 128], fp32)
            nc.vector.tensor_copy(rowF[:], rowI[:])
            ident = pp.tile([128, 128], fp32)
            nc.vector.tensor_scalar(
                out=ident[:], in0=rowF[:], scalar1=colF[:], scalar2=None,
                op0=Alu.is_equal,
            )
            onehotE = rp.tile([EL, EL], fp32r)
            nc.vector.tensor_copy(onehotE[:], ident[0:EL, 0:EL])

            # ------- input loads (already in SBUF layout; contiguous) -------
            xTf = rp.tile([128, HC, T], fp32)     # gate operand (f32)
            nc.sync.dma_start(xTf[:], XT.ap())
            gwT = rp.tile([128, HC, E], fp32)
            nc.sync.dma_start(gwT[:], GWT.ap())
            xTr = pp.tile([128, HC, T], fp32r)    # expert operand (f32r cast)
            nc.vector.tensor_copy(xTr[:], xTf[:])  # on-chip cast, saves 1MB DMA
            biasB = rp.tile([128, E], fp32)
            nc.sync.dma_start(
                biasB[:], EB.ap().unsqueeze(0).broadcast_to([128, E]))
            CB_all = pp.tile([128, EL, T], fp32)  # combine bcast (filled later)

            # ------- expert weights: contiguous up (2MB) + wd (1MB) DMAs ----
            wup, wdn = {}, {}

            def ensure_up_w(e):
                if e < EL and e not in wup:
                    wup[e] = wp.tile([128, 2, 2048], fp32r, tag="wu",
                                     name=f"wu{e}")
                    if e >= EL - 4:
                        # tail experts: split halves so the h1 matmuls start
                        # as soon as w1 lands, overlapping the w3 transfer
                        nc.gpsimd.dma_start(wup[e][:, 0, :], WBU.ap()[e][:, 0, :])
                        nc.gpsimd.dma_start(wup[e][:, 1, :], WBU.ap()[e][:, 1, :])
                    else:
                        nc.gpsimd.dma_start(wup[e][:], WBU.ap()[e])

            def ensure_wd_w(e):
                if e < EL and e not in wdn:
                    wdn[e] = wp.tile([128, 2048], fp32r, tag="wd",
                                     name=f"wdn{e}")
                    nc.gpsimd.dma_start(wdn[e][:], WBD.ap()[e])

            ensure_up_w(0)
            swg_t = pp.tile([128, HC, IS], fp32r)
            nc.gpsimd.dma_start(swg_t[:], SWGT.ap())
            swu_t = pp.tile([128, HC, IS], fp32r)
            nc.gpsimd.dma_start(swu_t[:], SWUT.ap())
            swd_t = pp.tile([IS, H], fp32r)
            nc.gpsimd.dma_start(swd_t[:], SWD.ap())
            for e in range(1, min(ahead, EL)):
                ensure_up_w(e)
            for e in range(max(0, ahead - 2)):
                ensure_wd_w(e)

            # ---------- routing (per token chunk) ----------
            combT = rp.tile([EL, T], fp32r)      # combine^T for local experts
            for t_c in range(TC):
                lg = tp.tile([128, 2, T], fp32, tag="ps")
                for hc in range(HC):
                    nc.tensor.matmul(
                        lg[:, 0, :], xTf[:, hc, t_c * 128:(t_c + 1) * 128],
                        gwT[:, hc, :], start=(hc == 0), stop=(hc == HC - 1),
                        skip_group_check=True)
                scores = rp.tile([128, E], fp32, tag="scores")
                nc.scalar.activation(scores[:], lg[:, 0, :], Act.Sigmoid)
                sc = rp.tile([128, E], fp32, tag="sc")
                nc.vector.tensor_tensor(
                    out=sc[:], in0=scores[:], in1=biasB[:], op=Alu.add)

                gs = rp.tile([128, N_GROUP], fp32, tag="gs")
                for g in range(N_GROUP):
                    g8 = rp.tile([128, 8], fp32, tag="g8")
                    nc.vector.max(g8[:], sc[:, g * 32:(g + 1) * 32])
                    nc.vector.reduce_sum(
                        gs[:, g:g + 1], g8[:, 0:2], axis=mybir.AxisListType.X)
                gs8 = rp.tile([128, 8], fp32, tag="gs8")
                nc.vector.max(gs8[:], gs[:])
                gmask = rp.tile([128, N_GROUP], fp32, tag="gmask")
                nc.vector.tensor_scalar(
                    out=gmask[:], in0=gs[:],
                    scalar1=gs8[:, TOPK_GROUP - 1:TOPK_GROUP],
                    scalar2=None, op0=Alu.is_ge)
                gpen = rp.tile([128, N_GROUP], fp32, tag="gpen")
                nc.vector.tensor_scalar(
                    out=gpen[:], in0=gmask[:], scalar1=1.0, scalar2=1e30,
                    op0=Alu.subtract, op1=Alu.mult)
                epen = rp.tile([128, E], fp32, tag="epen")
                nc.vector.tensor_copy(
                    epen[:].rearrange("p (g j) -> p g j", g=N_GROUP),
                    gpen[:].unsqueeze(2).broadcast_to([128, N_GROUP, 32]))
                masked = rp.tile([128, E], fp32, tag="masked")
                nc.vector.tensor_tensor(
                    out=masked[:], in0=sc[:], in1=epen[:], op=Alu.add)
                t8 = rp.tile([128, 8], fp32, tag="t8")
                nc.vector.max(t8[:], masked[:])
                sel = rp.tile([128, E], fp32, tag="sel")
                nc.vector.tensor_scalar(
                    out=sel[:], in0=masked[:],
                    scalar1=t8[:, TOP_K - 1:TOP_K],
                    scalar2=None, op0=Alu.is_ge)
                wsel = rp.tile([128, E], fp32, tag="epen", name="wsel")
                sw = rp.tile([128, 1], fp32, tag="sw")
                nc.vector.scalar_tensor_tensor(
                    out=wsel[:], in0=scores[:], scalar=1.0, in1=sel[:],
                    op0=Alu.mult, op1=Alu.mult, accum_out=sw[:])
                swp = rp.tile([128, 1], fp32, tag="swp")
                nc.vector.tensor_scalar(
                    out=swp[:], in0=sw[:], scalar1=1e-20, scalar2=None,
                    op0=Alu.add)
                rn = rp.tile([128, 1], fp32, tag="rn")
                nc.vector.reciprocal(rn[:], swp[:])
                comb = rp.tile([128, E], fp32, tag="scores", name="comb")
                nc.vector.tensor_scalar(
                    out=comb[:], in0=wsel[:], scalar1=rn[:], scalar2=SCALE,
                    op0=Alu.mult, op1=Alu.mult)
                ps_c = tp.tile([128, 2, T], fp32, tag="ps")
                nc.tensor.transpose(
                    ps_c[0:EL, 0, 0:128], comb[:, 0:EL], ident[:])
                nc.vector.tensor_copy(
                    combT[:, t_c * 128:(t_c + 1) * 128], ps_c[0:EL, 0, 0:128])

            # ---------- helpers: expert up-projection + activation ----------
            a13_t = {}

            def emit_up(e):
                ensure_up_w(e + ahead)
                ensure_wd_w(e + ahead - 2)
                hh = hp.tile([128, 2, IC, T], fp32, tag="hh", name=f"hh{e}")
                w = wup[e]
                for mi in range(IC):
                    for hc in range(HC):
                        nc.tensor.matmul(
                            hh[:, 0, mi, :],
                            w[:, 0, hc * I + mi * 128:hc * I + (mi + 1) * 128],
                            xTr[:, hc, :],
                            start=(mi == 0 and hc == 0), stop=(hc == HC - 1),
                            skip_group_check=True)
                for mi in range(IC):
                    for hc in range(HC):
                        nc.tensor.matmul(
                            hh[:, 1, mi, :],
                            w[:, 1, hc * I + mi * 128:hc * I + (mi + 1) * 128],
                            xTr[:, hc, :],
                            start=(mi == 0 and hc == 0), stop=(hc == HC - 1),
                            skip_group_check=True)
                s1 = s1p.tile([128, IC, T], fp32r, tag="s1", name=f"s1_{e}")
                nc.scalar.activation(s1[:], hh[:, 0, :, :], Act.Silu)
                a13 = a13p.tile([128, IC, T], fp32r, tag="a13", name=f"a13_{e}")
                nc.vector.tensor_tensor(
                    out=a13[:], in0=hh[:, 1, :, :], in1=s1[:], op=Alu.mult)
                a13_t[e] = a13

            # shared expert up-path (no routing dependency)
            hsu = hp.tile([IS, 2, IC, T], fp32, tag="hh")
            for hc in range(HC):
                nc.tensor.matmul(
                    hsu[:, 0, 0, :], swg_t[:, hc, :], xTr[:, hc, :],
                    start=(hc == 0), stop=(hc == HC - 1),
                    skip_group_check=True)
            for hc in range(HC):
                nc.tensor.matmul(
                    hsu[:, 1, 0, :], swu_t[:, hc, :], xTr[:, hc, :],
                    start=(hc == 0), stop=(hc == HC - 1),
                    skip_group_check=True)
            s_s1 = sp.tile([IS, T], fp32r, tag="ss1")
            nc.scalar.activation(s_s1[:], hsu[:, 0, 0, :], Act.Silu)
            s_act = sp.tile([IS, T], fp32r, tag="sact")
            nc.vector.tensor_tensor(
                out=s_act[:], in0=hsu[:, 1, 0, :], in1=s_s1[:], op=Alu.mult)

            # first experts' up-path keeps PE busy while routing DVE runs
            for e in range(pre_n):
                emit_up(e)

            # broadcast combT rows to all 128 partitions via PE:
            for j in range(EL // 2):
                cb_ps = tp.tile([128, 2, T], fp32, tag="ps")
                for h in range(2):
                    e = 2 * j + h
                    nc.tensor.matmul(
                        cb_ps[:, h, :],
                        onehotE[:, e:e + 1].broadcast_to([EL, 128]),
                        combT[:], start=True, stop=True,
                        skip_group_check=True)
                nc.scalar.copy(CB_all[:, 2 * j:2 * j + 2, :], cb_ps[:])

          # ---------- experts ----------
          with tc.tile_pool(name="ypsum", bufs=1, space="PSUM") as yp:
            y_ps = yp.tile([128, TC, H], fp32)   # Y[t, h] accumulator

            # shared expert down-projection first: only needs s_act, and
            # keeps it off the critical tail after the last expert
            for t_c in range(TC):
                for nh in range(2):
                    nc.tensor.matmul(
                        y_ps[:, t_c, nh * 512:(nh + 1) * 512],
                        s_act[:, t_c * 128:(t_c + 1) * 128],
                        swd_t[:, nh * 512:(nh + 1) * 512],
                        start=True, stop=False,
                        skip_group_check=True)

            for e in range(EL):
                if e >= pre_n:
                    emit_up(e)
                act_t = s1p.tile([128, IC, T], fp32r, tag="act", name=f"act{e}")
                nc.vector.tensor_tensor(
                    out=act_t[:], in0=a13_t.pop(e)[:],
                    in1=CB_all[:, e, :].unsqueeze(1).broadcast_to([128, IC, T]),
                    op=Alu.mult)

                wdv = wdn[e][:].rearrange("p (c h) -> p c h", c=IC)
                for t_c in range(TC):
                    for nh in range(2):
                        for ic in range(IC):
                            nc.tensor.matmul(
                                y_ps[:, t_c, nh * 512:(nh + 1) * 512],
                                act_t[:, ic, t_c * 128:(t_c + 1) * 128],
                                wdv[:, ic, nh * 512:(nh + 1) * 512],
                                start=False,
                                stop=(e == EL - 1 and ic == IC - 1),
                                skip_group_check=True)

            # ---------- copy out (+ AllReduce in multi-core) ----------
            if single_core:
                for t_c in range(TC):
                    for half in range(2):
                        k = 2 * t_c + half
                        yo = s1p.tile([128, 512], fp32,
                                      tag=("act" if k % 2 == 0 else "s1"),
                                      name=f"yo{t_c}_{half}")
                        if k % 2 == 0:
                            nc.vector.tensor_copy(
                                yo[:], y_ps[:, t_c, half * 512:(half + 1) * 512])
                        else:
                            nc.scalar.copy(
                                yo[:], y_ps[:, t_c, half * 512:(half + 1) * 512])
                        nc.sync.dma_start(
                            Y.ap()[t_c * 128:(t_c + 1) * 128,
                                   half * 512:(half + 1) * 512], yo[:])
            else:
                in_b = dp.tile([T, H], fp32)
                out_b = dp.tile([T, H], fp32, addr_space="Shared")
                for t_c in range(TC):
                    for half in range(2):
                        k = 2 * t_c + half
                        yo = s1p.tile([128, 512], fp32,
                                      tag=("act" if k % 2 == 0 else "s1"),
                                      name=f"yo{t_c}_{half}")
                        if k % 2 == 0:
                            nc.vector.tensor_copy(
                                yo[:], y_ps[:, t_c, half * 512:(half + 1) * 512])
                        else:
                            nc.scalar.copy(
                                yo[:], y_ps[:, t_c, half * 512:(half + 1) * 512])
                        nc.sync.dma_start(
                            in_b[t_c * 128:(t_c + 1) * 128,
                                 half * 512:(half + 1) * 512], yo[:])
                nc.gpsimd.collective_compute(
                    "AllReduce", Alu.add,
                    replica_groups=[list(range(N_CORES))],
                    ins=[in_b.opt()], outs=[out_b.opt()])
                nc.sync.dma_start(Y.ap(), out_b[:])

    nc.finalize()
    return nc


def _get_nc():
    if "nc" not in _NC_CACHE:
        _NC_CACHE["nc"] = build_nc()
    return _NC_CACHE["nc"]


def _sw(a):
    """[X, HC*128] -> [128, HC, X]-style partition-major swizzle."""
    n, h = a.shape
    return np.ascontiguousarray(a.reshape(n, HC, 128).transpose(2, 1, 0))


def make_in_maps(inputs):
    x = np.asarray(inputs["hidden_states"], dtype=np.float32).reshape(T, H)
    gate_w = np.asarray(inputs["gate_w"], dtype=np.float32)
    e_bias = np.asarray(inputs["e_bias"], dtype=np.float32)
    w_gate = np.asarray(inputs["w_gate"], dtype=np.float32)
    w_up = np.asarray(inputs["w_up"], dtype=np.float32)
    w_down = np.asarray(inputs["w_down"], dtype=np.float32)
    sw_gate = np.asarray(inputs["sw_gate"], dtype=np.float32)
    sw_up = np.asarray(inputs["sw_up"], dtype=np.float32)
    sw_down = np.asarray(inputs["sw_down"], dtype=np.float32)

    xt = _sw(x)  # [128, HC, T]
    in_maps = []
    for c in range(N_CORES):
        order = [(c + k) % N_GROUP for k in range(N_GROUP)]
        perm = np.concatenate([np.arange(g * EL, (g + 1) * EL) for g in order])
        sl = slice(c * EL, (c + 1) * EL)
        # per-expert packs, partition-major
        bu = np.empty((EL, 128, 2, 2048), np.float32)
        bu[:, :, 0, :] = w_gate[sl].reshape(EL, HC, 128, I).transpose(
            0, 2, 1, 3).reshape(EL, 128, HC * I)
        bu[:, :, 1, :] = w_up[sl].reshape(EL, HC, 128, I).transpose(
            0, 2, 1, 3).reshape(EL, 128, HC * I)
        bd = np.ascontiguousarray(
            w_down[sl].reshape(EL, IC, 128, H).transpose(
                0, 2, 1, 3).reshape(EL, 128, IC * H))
        in_maps.append({
            "xt": xt,
            "gwt": _sw(np.ascontiguousarray(gate_w[perm])),
            "ebp": np.ascontiguousarray(e_bias[perm]),
            "wbu": bu,
            "wbd": bd,
            "swgt": _sw(np.ascontiguousarray(sw_gate[:, c * IS:(c + 1) * IS].T)),
            "swut": _sw(np.ascontiguousarray(sw_up[:, c * IS:(c + 1) * IS].T)),
            "swd": np.ascontiguousarray(sw_down[c * IS:(c + 1) * IS, :]),
        })
    return in_maps


def kernel(**inputs) -> np.ndarray:
    nc = _get_nc()
    in_maps = make_in_maps(inputs)
    res = run_bass_kernel_spmd(nc, in_maps, core_ids=list(range(N_CORES)))
    y = res.results[0]["y"]
    return np.asarray(y, dtype=np.float32).reshape(1, 1, T, H)


if __name__ == "__main__":
    rng = np.random.default_rng(0)
    demo = {
        "hidden_states": rng.standard_normal((1, 1, T, H)).astype(np.float32),
        "gate_w": (rng.standard_normal((E, H)) / np.sqrt(H)).astype(np.float32),
        "e_bias": (rng.standard_normal(E) * 0.1).astype(np.float32),
        "w_gate": (rng.standard_normal((E, H, I)) / np.sqrt(H)).astype(np.float32),
        "w_up": (rng.standard_normal((E, H, I)) / np.sqrt(H)).astype(np.float32),
        "w_down": (rng.standard_normal((E, I, H)) / np.sqrt(I)).astype(np.float32),
        "sw_gate": (rng.standard_normal((H, I)) / np.sqrt(H)).astype(np.float32),
        "sw_up": (rng.standard_normal((H, I)) / np.sqrt(H)).astype(np.float32),
        "sw_down": (rng.standard_normal((I, H)) / np.sqrt(I)).astype(np.float32),
    }
    out = kernel(**demo)
    print("kernel output:", out.shape, out.dtype, np.abs(out).max())


# revision 39
# speedup vs baseline: 1.0542x; 1.0542x over previous
"""DeepSeek-V3 MoE routing kernel for 8x Trainium2 NeuronCores.

Strategy (expert-parallel, dense-per-core):
- 256 experts sharded 32/core. Gate (sigmoid + grouped top-k routing) is
  replicated on every core; per-core inputs are group-rotated so each core's
  32 local experts always occupy combine columns 0..31 (SPMD-friendly).
- Each core computes all 256 tokens through its 32 experts (dense), scales
  the intermediate activations by the combine weights, and accumulates the
  down-projections of all its experts (plus a 32-wide slice of the shared
  expert) directly in PSUM. Partial outputs are summed with an AllReduce.
- Expert matmuls run in float32r (reduced-precision fp32, full PE rate);
  the gate matmul runs in full fp32 so top-k decisions match the reference.
- Expert weights stream from HBM in 2-expert (2 MB) SWDGE DMAs that cast
  f32 -> f32r in flight; this streaming is the bottleneck resource.

PSUM budget (8 banks): Y accumulator 4 + h1h3 double-buffer 2 + routing 2.
"""
import numpy as np

from concourse import bacc, tile
import concourse.mybir as mybir
from concourse.bass_utils import run_bass_kernel_spmd

E = 256
H = 1024
I = 256
T = 256
N_GROUP = 8
TOPK_GROUP = 4
TOP_K = 8
SCALE = 2.5
N_CORES = 8
EL = E // N_CORES          # local experts per core (= one routing group)
IS = I // N_CORES          # shared-expert intermediate slice per core
HC = H // 128              # h chunks
TC = T // 128              # token chunks
IC = I // 128              # intermediate chunks

fp32 = mybir.dt.float32
fp32r = mybir.dt.float32r
fp16 = mybir.dt.float16
i32 = mybir.dt.int32
Alu = mybir.AluOpType
Act = mybir.ActivationFunctionType

_NC_CACHE = {}


def build_nc(single_core=False, w_bufs=4, ahead=4, pre_n=4):
    # w_bufs applies to both the up (16KB) and wd (8KB) tags
    nc = bacc.Bacc("TRN2", debug=False, num_devices=1 if single_core else N_CORES)

    # host passes pre-swizzled layouts (pure layout transforms, no compute):
    #  xt   [128, HC, T]   : xt[p, c, t] = x[t, c*128+p]
    #  gwt  [128, HC, E]   : gwt[p, c, e] = gate_w_perm[e, c*128+p]
    #  wblob[EL, 128, 3, 2048]: per expert, partition-major pack of
    #       w1 (hc, i), w3 (hc, i), wd (ic, h)
    #  swgt/swut [128, HC, IS]; swd [IS, H]
    XT = nc.dram_tensor("xt", [128, HC, T], fp32, kind="ExternalInput")
    GWT = nc.dram_tensor("gwt", [128, HC, E], fp32, kind="ExternalInput")
    EB = nc.dram_tensor("ebp", [E], fp32, kind="ExternalInput")
    WBU = nc.dram_tensor("wbu", [EL, 128, 2, 2048], fp16, kind="ExternalInput")
    WBD = nc.dram_tensor("wbd", [EL, 128, 2048], fp16, kind="ExternalInput")
    SWGT = nc.dram_tensor("swgt", [128, HC, IS], fp16, kind="ExternalInput")
    SWUT = nc.dram_tensor("swut", [128, HC, IS], fp16, kind="ExternalInput")
    SWD = nc.dram_tensor("swd", [IS, H], fp16, kind="ExternalInput")
    Y = nc.dram_tensor("y", [T, H], fp32, kind="ExternalOutput")

    with tile.TileContext(nc) as tc:
        with (
            tc.tile_pool(name="persist", bufs=1) as pp,
            tc.tile_pool(name="route", bufs=1) as rp,
            tc.tile_pool(name="wpool", bufs=w_bufs) as wp,
            tc.tile_pool(name="spool", bufs=2) as sp,
            tc.tile_pool(name="s1pool", bufs=1) as s1p,
            tc.tile_pool(name="a13pool", bufs=5) as a13p,
            tc.tile_pool(name="hpsum", bufs=2, space="PSUM") as hp,
            tc.tile_pool(name="dram", bufs=1, space="DRAM") as dp,
        ):
          with tc.tile_pool(name="tpsum", bufs=3, space="PSUM") as tp:
            # tiny identity (for the combine transpose) built on DVE/Pool
            colI = rp.tile([128, 1], i32)
            nc.gpsimd.iota(colI[:], [[0, 1]], channel_multiplier=1, base=0)
            colF = rp.tile([128, 1], fp32)
            nc.vector.tensor_copy(colF[:], colI[:])
            rowI = rp.tile([128, 128], i32)
            nc.gpsimd.iota(rowI[:], [[1, 128]], channel_multiplier=0, base=0)
            rowF = rp.tile([128, 128], fp32)
            nc.vector.tensor_copy(rowF[:], rowI[:])
            ident = pp.tile([128, 128], fp32)
            nc.vector.tensor_scalar(
                out=ident[:], in0=rowF[:], scalar1=colF[:], scalar2=None,
                op0=Alu.is_equal,
            )
            onehotE = rp.tile([EL, EL], fp32r)
            nc.vector.tensor_copy(onehotE[:], ident[0:EL, 0:EL])

            # ------- input loads (already in SBUF layout; contiguous) -------
            xTf = rp.tile([128, HC, T], fp32)     # gate operand (f32)
            nc.sync.dma_start(xTf[:], XT.ap())
            gwT = rp.tile([128, HC, E], fp32)
            nc.sync.dma_start(gwT[:], GWT.ap())
            xTr = pp.tile([128, HC, T], fp16)     # expert operand (fp16 cast)
            nc.vector.tensor_copy(xTr[:], xTf[:])  # on-chip cast, saves 1MB DMA
            biasB = rp.tile([128, E], fp32)
            nc.sync.dma_start(
                biasB[:], EB.ap().unsqueeze(0).broadcast_to([128, E]))
            CB_all = pp.tile([128, EL, T], fp32)  # combine bcast (filled later)

            # ------- expert weights: contiguous up (2MB) + wd (1MB) DMAs ----
            wup, wdn = {}, {}

            def ensure_up_w(e):
                if e < EL and e not in wup:
                    wup[e] = wp.tile([128, 2, 2048], fp16, tag="wu",
                                     name=f"wu{e}")
                    if e >= EL - 4:
                        # tail experts: split halves so the h1 matmuls start
                        # as soon as w1 lands, overlapping the w3 transfer
                        nc.sync.dma_start(wup[e][:, 0, :], WBU.ap()[e][:, 0, :])
                        nc.sync.dma_start(wup[e][:, 1, :], WBU.ap()[e][:, 1, :])
                    else:
                        nc.sync.dma_start(wup[e][:], WBU.ap()[e])

            def ensure_wd_w(e):
                if e < EL and e not in wdn:
                    wdn[e] = wp.tile([128, 2048], fp16, tag="wd",
                                     name=f"wdn{e}")
                    nc.scalar.dma_start(wdn[e][:], WBD.ap()[e])

            ensure_up_w(0)
            swg_t = pp.tile([128, HC, IS], fp16)
            nc.sync.dma_start(swg_t[:], SWGT.ap())
            swu_t = pp.tile([128, HC, IS], fp16)
            nc.sync.dma_start(swu_t[:], SWUT.ap())
            swd_t = pp.tile([IS, H], fp16)
            nc.sync.dma_start(swd_t[:], SWD.ap())
            for e in range(1, min(ahead, EL)):
                ensure_up_w(e)
            for e in range(max(0, ahead - 2)):
                ensure_wd_w(e)

            # ---------- routing (per token chunk) ----------
            combT = rp.tile([EL, T], fp32r)      # combine^T for local experts
            for t_c in range(TC):
                lg = tp.tile([128, 2, T], fp32, tag="ps")
                for hc in range(HC):
                    nc.tensor.matmul(
                        lg[:, 0, :], xTf[:, hc, t_c * 128:(t_c + 1) * 128],
                        gwT[:, hc, :], start=(hc == 0), stop=(hc == HC - 1),
                        skip_group_check=True)
                scores = rp.tile([128, E], fp32, tag="scores")
                nc.scalar.activation(scores[:], lg[:, 0, :], Act.Sigmoid)
                sc = rp.tile([128, E], fp32, tag="sc")
                nc.vector.tensor_tensor(
                    out=sc[:], in0=scores[:], in1=biasB[:], op=Alu.add)

                gs = rp.tile([128, N_GROUP], fp32, tag="gs")
                for g in range(N_GROUP):
                    g8 = rp.tile([128, 8], fp32, tag="g8")
                    nc.vector.max(g8[:], sc[:, g * 32:(g + 1) * 32])
                    nc.vector.reduce_sum(
                        gs[:, g:g + 1], g8[:, 0:2], axis=mybir.AxisListType.X)
                gs8 = rp.tile([128, 8], fp32, tag="gs8")
                nc.vector.max(gs8[:], gs[:])
                gmask = rp.tile([128, N_GROUP], fp32, tag="gmask")
                nc.vector.tensor_scalar(
                    out=gmask[:], in0=gs[:],
                    scalar1=gs8[:, TOPK_GROUP - 1:TOPK_GROUP],
                    scalar2=None, op0=Alu.is_ge)
                gpen = rp.tile([128, N_GROUP], fp32, tag="gpen")
                nc.vector.tensor_scalar(
                    out=gpen[:], in0=gmask[:], scalar1=1.0, scalar2=1e30,
                    op0=Alu.subtract, op1=Alu.mult)
                epen = rp.tile([128, E], fp32, tag="epen")
                nc.vector.tensor_copy(
                    epen[:].rearrange("p (g j) -> p g j", g=N_GROUP),
                    gpen[:].unsqueeze(2).broadcast_to([128, N_GROUP, 32]))
                masked = rp.tile([128, E], fp32, tag="masked")
                nc.vector.tensor_tensor(
                    out=masked[:], in0=sc[:], in1=epen[:], op=Alu.add)
                t8 = rp.tile([128, 8], fp32, tag="t8")
                nc.vector.max(t8[:], masked[:])
                sel = rp.tile([128, E], fp32, tag="sel")
                nc.vector.tensor_scalar(
                    out=sel[:], in0=masked[:],
                    scalar1=t8[:, TOP_K - 1:TOP_K],
                    scalar2=None, op0=Alu.is_ge)
                wsel = rp.tile([128, E], fp32, tag="epen", name="wsel")
                sw = rp.tile([128, 1], fp32, tag="sw")
                nc.vector.scalar_tensor_tensor(
                    out=wsel[:], in0=scores[:], scalar=1.0, in1=sel[:],
                    op0=Alu.mult, op1=Alu.mult, accum_out=sw[:])
                swp = rp.tile([128, 1], fp32, tag="swp")
                nc.vector.tensor_scalar(
                    out=swp[:], in0=sw[:], scalar1=1e-20, scalar2=None,
                    op0=Alu.add)
                rn = rp.tile([128, 1], fp32, tag="rn")
                nc.vector.reciprocal(rn[:], swp[:])
                comb = rp.tile([128, E], fp32, tag="scores", name="comb")
                nc.vector.tensor_scalar(
                    out=comb[:], in0=wsel[:], scalar1=rn[:], scalar2=SCALE,
                    op0=Alu.mult, op1=Alu.mult)
                ps_c = tp.tile([128, 2, T], fp32, tag="ps")
                nc.tensor.transpose(
                    ps_c[0:EL, 0, 0:128], comb[:, 0:EL], ident[:])
                nc.vector.tensor_copy(
                    combT[:, t_c * 128:(t_c + 1) * 128], ps_c[0:EL, 0, 0:128])

            # ---------- helpers: expert up-projection + activation ----------
            a13_t = {}

            def emit_up(e):
                ensure_up_w(e + ahead)
                ensure_wd_w(e + ahead - 2)
                hh = hp.tile([128, 2, IC, T], fp32, tag="hh", name=f"hh{e}")
                w = wup[e]
                for mi in range(IC):
                    for hc in range(HC):
                        nc.tensor.matmul(
                            hh[:, 0, mi, :],
                            w[:, 0, hc * I + mi * 128:hc * I + (mi + 1) * 128],
                            xTr[:, hc, :],
                            start=(mi == 0 and hc == 0), stop=(hc == HC - 1),
                            skip_group_check=True)
                for mi in range(IC):
                    for hc in range(HC):
                        nc.tensor.matmul(
                            hh[:, 1, mi, :],
                            w[:, 1, hc * I + mi * 128:hc * I + (mi + 1) * 128],
                            xTr[:, hc, :],
                            start=(mi == 0 and hc == 0), stop=(hc == HC - 1),
                            skip_group_check=True)
                s1 = s1p.tile([128, IC, T], fp16, tag="s1", name=f"s1_{e}")
                nc.scalar.activation(s1[:], hh[:, 0, :, :], Act.Silu)
                a13 = a13p.tile([128, IC, T], fp16, tag="a13", name=f"a13_{e}")
                nc.vector.tensor_tensor(
                    out=a13[:], in0=hh[:, 1, :, :], in1=s1[:], op=Alu.mult)
                a13_t[e] = a13

            # shared expert up-path (no routing dependency)
            hsu = hp.tile([IS, 2, IC, T], fp32, tag="hh")
            for hc in range(HC):
                nc.tensor.matmul(
                    hsu[:, 0, 0, :], swg_t[:, hc, :], xTr[:, hc, :],
                    start=(hc == 0), stop=(hc == HC - 1),
                    skip_group_check=True)
            for hc in range(HC):
                nc.tensor.matmul(
                    hsu[:, 1, 0, :], swu_t[:, hc, :], xTr[:, hc, :],
                    start=(hc == 0), stop=(hc == HC - 1),
                    skip_group_check=True)
            s_s1 = sp.tile([IS, T], fp16, tag="ss1")
            nc.scalar.activation(s_s1[:], hsu[:, 0, 0, :], Act.Silu)
            s_act = sp.tile([IS, T], fp16, tag="sact")
            nc.vector.tensor_tensor(
                out=s_act[:], in0=hsu[:, 1, 0, :], in1=s_s1[:], op=Alu.mult)

            # first experts' up-path keeps PE busy while routing DVE runs
            for e in range(pre_n):
                emit_up(e)

            # broadcast combT rows to all 128 partitions via PE:
            for j in range(EL // 2):
                cb_ps = tp.tile([128, 2, T], fp32, tag="ps")
                for h in range(2):
                    e = 2 * j + h
                    nc.tensor.matmul(
                        cb_ps[:, h, :],
                        onehotE[:, e:e + 1].broadcast_to([EL, 128]),
                        combT[:], start=True, stop=True,
                        skip_group_check=True)
                nc.scalar.copy(CB_all[:, 2 * j:2 * j + 2, :], cb_ps[:])

          # ---------- experts ----------
          with tc.tile_pool(name="ypsum", bufs=1, space="PSUM") as yp:
            y_ps = yp.tile([128, TC, H], fp32)   # Y[t, h] accumulator

            # shared expert down-projection first: only needs s_act, and
            # keeps it off the critical tail after the last expert
            for t_c in range(TC):
                for nh in range(2):
                    nc.tensor.matmul(
                        y_ps[:, t_c, nh * 512:(nh + 1) * 512],
                        s_act[:, t_c * 128:(t_c + 1) * 128],
                        swd_t[:, nh * 512:(nh + 1) * 512],
                        start=True, stop=False,
                        skip_group_check=True)

            for e in range(EL):
                if e >= pre_n:
                    emit_up(e)
                act_t = s1p.tile([128, IC, T], fp16, tag="act", name=f"act{e}")
                nc.vector.tensor_tensor(
                    out=act_t[:], in0=a13_t.pop(e)[:],
                    in1=CB_all[:, e, :].unsqueeze(1).broadcast_to([128, IC, T]),
                    op=Alu.mult)

                wdv = wdn[e][:].rearrange("p (c h) -> p c h", c=IC)
                for t_c in range(TC):
                    for nh in range(2):
                        for ic in range(IC):
                            nc.tensor.matmul(
                                y_ps[:, t_c, nh * 512:(nh + 1) * 512],
                                act_t[:, ic, t_c * 128:(t_c + 1) * 128],
                                wdv[:, ic, nh * 512:(nh + 1) * 512],
                                start=False,
                                stop=(e == EL - 1 and ic == IC - 1),
                                skip_group_check=True)

            # ---------- copy out (+ AllReduce in multi-core) ----------
            if single_core:
                for t_c in range(TC):
                    for half in range(2):
                        k = 2 * t_c + half
                        yo = s1p.tile([128, 512], fp32,
                                      tag=("act" if k % 2 == 0 else "s1"),
                                      name=f"yo{t_c}_{half}")
                        if k % 2 == 0:
                            nc.vector.tensor_copy(
                                yo[:], y_ps[:, t_c, half * 512:(half + 1) * 512])
                        else:
                            nc.scalar.copy(
                                yo[:], y_ps[:, t_c, half * 512:(half + 1) * 512])
                        nc.sync.dma_start(
                            Y.ap()[t_c * 128:(t_c + 1) * 128,
                                   half * 512:(half + 1) * 512], yo[:])
            else:
                in_b = dp.tile([T, H], fp32)
                out_b = dp.tile([T, H], fp32, addr_space="Shared")
                for t_c in range(TC):
                    for half in range(2):
                        k = 2 * t_c + half
                        yo = s1p.tile([128, 512], fp32,
                                      tag=("act" if k % 2 == 0 else "s1"),
                                      name=f"yo{t_c}_{half}")
                        if k % 2 == 0:
                            nc.vector.tensor_copy(
                                yo[:], y_ps[:, t_c, half * 512:(half + 1) * 512])
                        else:
                            nc.scalar.copy(
                                yo[:], y_ps[:, t_c, half * 512:(half + 1) * 512])
                        nc.sync.dma_start(
                            in_b[t_c * 128:(t_c + 1) * 128,
                                 half * 512:(half + 1) * 512], yo[:])
                nc.gpsimd.collective_compute(
                    "AllReduce", Alu.add,
                    replica_groups=[list(range(N_CORES))],
                    ins=[in_b.opt()], outs=[out_b.opt()])
                nc.sync.dma_start(Y.ap(), out_b[:])

    nc.finalize()
    return nc


def _get_nc():
    if "nc" not in _NC_CACHE:
        _NC_CACHE["nc"] = build_nc()
    return _NC_CACHE["nc"]


def _sw(a):
    """[X, HC*128] -> [128, HC, X]-style partition-major swizzle."""
    n, h = a.shape
    return np.ascontiguousarray(a.reshape(n, HC, 128).transpose(2, 1, 0))


def make_in_maps(inputs):
    x = np.asarray(inputs["hidden_states"], dtype=np.float32).reshape(T, H)
    gate_w = np.asarray(inputs["gate_w"], dtype=np.float32)
    e_bias = np.asarray(inputs["e_bias"], dtype=np.float32)
    w_gate = np.asarray(inputs["w_gate"], dtype=np.float32)
    w_up = np.asarray(inputs["w_up"], dtype=np.float32)
    w_down = np.asarray(inputs["w_down"], dtype=np.float32)
    sw_gate = np.asarray(inputs["sw_gate"], dtype=np.float32)
    sw_up = np.asarray(inputs["sw_up"], dtype=np.float32)
    sw_down = np.asarray(inputs["sw_down"], dtype=np.float32)

    xt = _sw(x)  # [128, HC, T]
    in_maps = []
    for c in range(N_CORES):
        order = [(c + k) % N_GROUP for k in range(N_GROUP)]
        perm = np.concatenate([np.arange(g * EL, (g + 1) * EL) for g in order])
        sl = slice(c * EL, (c + 1) * EL)
        # per-expert packs, partition-major
        bu = np.empty((EL, 128, 2, 2048), np.float32)
        bu[:, :, 0, :] = w_gate[sl].reshape(EL, HC, 128, I).transpose(
            0, 2, 1, 3).reshape(EL, 128, HC * I)
        bu[:, :, 1, :] = w_up[sl].reshape(EL, HC, 128, I).transpose(
            0, 2, 1, 3).reshape(EL, 128, HC * I)
        bd = np.ascontiguousarray(
            w_down[sl].reshape(EL, IC, 128, H).transpose(
                0, 2, 1, 3).reshape(EL, 128, IC * H))
        in_maps.append({
            "xt": xt,
            "gwt": _sw(np.ascontiguousarray(gate_w[perm])),
            "ebp": np.ascontiguousarray(e_bias[perm]),
            "wbu": bu.astype(np.float16),
            "wbd": bd.astype(np.float16),
            "swgt": _sw(np.ascontiguousarray(
                sw_gate[:, c * IS:(c + 1) * IS].T)).astype(np.float16),
            "swut": _sw(np.ascontiguousarray(
                sw_up[:, c * IS:(c + 1) * IS].T)).astype(np.float16),
            "swd": np.ascontiguousarray(
                sw_down[c * IS:(c + 1) * IS, :]).astype(np.float16),
        })
    return in_maps


def kernel(**inputs) -> np.ndarray:
    nc = _get_nc()
    in_maps = make_in_maps(inputs)
    res = run_bass_kernel_spmd(nc, in_maps, core_ids=list(range(N_CORES)))
    y = res.results[0]["y"]
    return np.asarray(y, dtype=np.float32).reshape(1, 1, T, H)


if __name__ == "__main__":
    rng = np.random.default_rng(0)
    demo = {
        "hidden_states": rng.standard_normal((1, 1, T, H)).astype(np.float32),
        "gate_w": (rng.standard_normal((E, H)) / np.sqrt(H)).astype(np.float32),
        "e_bias": (rng.standard_normal(E) * 0.1).astype(np.float32),
        "w_gate": (rng.standard_normal((E, H, I)) / np.sqrt(H)).astype(np.float32),
        "w_up": (rng.standard_normal((E, H, I)) / np.sqrt(H)).astype(np.float32),
        "w_down": (rng.standard_normal((E, I, H)) / np.sqrt(I)).astype(np.float32),
        "sw_gate": (rng.standard_normal((H, I)) / np.sqrt(H)).astype(np.float32),
        "sw_up": (rng.standard_normal((H, I)) / np.sqrt(H)).astype(np.float32),
        "sw_down": (rng.standard_normal((I, H)) / np.sqrt(I)).astype(np.float32),
    }
    out = kernel(**demo)
    print("kernel output:", out.shape, out.dtype, np.abs(out).max())


# revision 44
# speedup vs baseline: 1.2522x; 1.1878x over previous
"""DeepSeek-V3 MoE routing kernel for 8x Trainium2 NeuronCores.

Strategy (expert-parallel, dense-per-core):
- 256 experts sharded 32/core. Gate (sigmoid + grouped top-k routing) is
  replicated on every core; per-core inputs are group-rotated so each core's
  32 local experts always occupy combine columns 0..31 (SPMD-friendly).
- Each core computes all 256 tokens through its 32 experts (dense), scales
  the intermediate activations by the combine weights, and accumulates the
  down-projections of all its experts (plus a 32-wide slice of the shared
  expert) directly in PSUM. Partial outputs are summed with an AllReduce.
- Expert matmuls run in float32r (reduced-precision fp32, full PE rate);
  the gate matmul runs in full fp32 so top-k decisions match the reference.
- Expert weights stream from HBM in 2-expert (2 MB) SWDGE DMAs that cast
  f32 -> f32r in flight; this streaming is the bottleneck resource.

PSUM budget (8 banks): Y accumulator 4 + h1h3 double-buffer 2 + routing 2.
"""
import numpy as np

from concourse import bacc, tile
import concourse.mybir as mybir
from concourse.bass_utils import run_bass_kernel_spmd

E = 256
H = 1024
I = 256
T = 256
N_GROUP = 8
TOPK_GROUP = 4
TOP_K = 8
SCALE = 2.5
N_CORES = 8
EL = E // N_CORES          # local experts per core (= one routing group)
IS = I // N_CORES          # shared-expert intermediate slice per core
HC = H // 128              # h chunks
TC = T // 128              # token chunks
IC = I // 128              # intermediate chunks

fp32 = mybir.dt.float32
fp32r = mybir.dt.float32r
fp16 = mybir.dt.float16
i32 = mybir.dt.int32
Alu = mybir.AluOpType
Act = mybir.ActivationFunctionType

_NC_CACHE = {}


def build_nc(single_core=False, w_bufs=4, ahead=4, pre_n=4):
    # w_bufs applies to both the up (16KB) and wd (8KB) tags
    nc = bacc.Bacc("TRN2", debug=False, num_devices=1 if single_core else N_CORES)

    # host passes pre-swizzled layouts (pure layout transforms, no compute):
    #  xt   [128, HC, T]   : xt[p, c, t] = x[t, c*128+p]
    #  gwt  [128, HC, E]   : gwt[p, c, e] = gate_w_perm[e, c*128+p]
    #  wblob[EL, 128, 3, 2048]: per expert, partition-major pack of
    #       w1 (hc, i), w3 (hc, i), wd (ic, h)
    #  swgt/swut [128, HC, IS]; swd [IS, H]
    XT = nc.dram_tensor("xt", [128, HC, T], fp32, kind="ExternalInput")
    GWT = nc.dram_tensor("gwt", [128, HC, E], fp32, kind="ExternalInput")
    EB = nc.dram_tensor("ebp", [E], fp32, kind="ExternalInput")
    WBU = nc.dram_tensor("wbu", [EL, 128, 2, 2048], fp16, kind="ExternalInput")
    WBD = nc.dram_tensor("wbd", [EL, 128, 2048], fp16, kind="ExternalInput")
    SWGT = nc.dram_tensor("swgt", [128, HC, IS], fp16, kind="ExternalInput")
    SWUT = nc.dram_tensor("swut", [128, HC, IS], fp16, kind="ExternalInput")
    SWD = nc.dram_tensor("swd", [IS, H], fp16, kind="ExternalInput")
    Y = nc.dram_tensor("y", [T, H], fp32, kind="ExternalOutput")

    with tile.TileContext(nc) as tc:
        with (
            tc.tile_pool(name="persist", bufs=1) as pp,
            tc.tile_pool(name="route", bufs=1) as rp,
            tc.tile_pool(name="wpool", bufs=w_bufs) as wp,
            tc.tile_pool(name="spool", bufs=2) as sp,
            tc.tile_pool(name="s1pool", bufs=1) as s1p,
            tc.tile_pool(name="a13pool", bufs=5) as a13p,
            tc.tile_pool(name="hpsum", bufs=2, space="PSUM") as hp,
            tc.tile_pool(name="dram", bufs=1, space="DRAM") as dp,
        ):
          with tc.tile_pool(name="tpsum", bufs=3, space="PSUM") as tp:
            # tiny identity (for the combine transpose) built on DVE/Pool
            colI = rp.tile([128, 1], i32)
            nc.gpsimd.iota(colI[:], [[0, 1]], channel_multiplier=1, base=0)
            colF = rp.tile([128, 1], fp32)
            nc.vector.tensor_copy(colF[:], colI[:])
            rowI = rp.tile([128, 128], i32)
            nc.gpsimd.iota(rowI[:], [[1, 128]], channel_multiplier=0, base=0)
            rowF = rp.tile([128, 128], fp32)
            nc.vector.tensor_copy(rowF[:], rowI[:])
            ident = pp.tile([128, 128], fp32)
            nc.vector.tensor_scalar(
                out=ident[:], in0=rowF[:], scalar1=colF[:], scalar2=None,
                op0=Alu.is_equal,
            )
            onehotE = rp.tile([EL, EL], fp32r)
            nc.vector.tensor_copy(onehotE[:], ident[0:EL, 0:EL])

            # ------- input loads (already in SBUF layout; contiguous) -------
            xTf = rp.tile([128, HC, T], fp32)     # gate operand (f32)
            nc.sync.dma_start(xTf[:], XT.ap())
            gwT = rp.tile([128, HC, E], fp32)
            nc.sync.dma_start(gwT[:], GWT.ap())
            xTr = pp.tile([128, HC, T], fp16)     # expert operand (fp16 cast)
            nc.vector.tensor_copy(xTr[:], xTf[:])  # on-chip cast, saves 1MB DMA
            biasB = rp.tile([128, E], fp32)
            nc.sync.dma_start(
                biasB[:], EB.ap().unsqueeze(0).broadcast_to([128, E]))
            CB_all = pp.tile([128, EL, T], fp32)  # combine bcast (filled later)

            # ------- expert weights: contiguous up (2MB) + wd (1MB) DMAs ----
            wup, wdn = {}, {}

            def ensure_up_w(e):
                if e < EL and e not in wup:
                    wup[e] = wp.tile([128, 2, 2048], fp16, tag="wu",
                                     name=f"wu{e}")
                    if e >= EL - 4:
                        # tail experts: split halves so the h1 matmuls start
                        # as soon as w1 lands, overlapping the w3 transfer
                        nc.sync.dma_start(wup[e][:, 0, :], WBU.ap()[e][:, 0, :])
                        nc.sync.dma_start(wup[e][:, 1, :], WBU.ap()[e][:, 1, :])
                    else:
                        nc.sync.dma_start(wup[e][:], WBU.ap()[e])

            def ensure_wd_w(e):
                if e < EL and e not in wdn:
                    wdn[e] = wp.tile([128, 2048], fp16, tag="wd",
                                     name=f"wdn{e}")
                    nc.scalar.dma_start(wdn[e][:], WBD.ap()[e])

            ensure_up_w(0)
            swg_t = pp.tile([128, HC, IS], fp16)
            nc.sync.dma_start(swg_t[:], SWGT.ap())
            swu_t = pp.tile([128, HC, IS], fp16)
            nc.sync.dma_start(swu_t[:], SWUT.ap())
            swd_t = pp.tile([IS, H], fp16)
            nc.sync.dma_start(swd_t[:], SWD.ap())
            for e in range(1, min(ahead, EL)):
                ensure_up_w(e)
            for e in range(max(0, ahead - 2)):
                ensure_wd_w(e)

            # ---------- routing (per token chunk) ----------
            combT = rp.tile([EL, T], fp32r)      # combine^T for local experts
            for t_c in range(TC):
                lg = tp.tile([128, 2, T], fp32, tag="ps")
                for hc in range(HC):
                    nc.tensor.matmul(
                        lg[:, 0, :], xTf[:, hc, t_c * 128:(t_c + 1) * 128],
                        gwT[:, hc, :], start=(hc == 0), stop=(hc == HC - 1),
                        skip_group_check=True)
                scores = rp.tile([128, E], fp32, tag="scores")
                nc.scalar.activation(scores[:], lg[:, 0, :], Act.Sigmoid)
                sc = rp.tile([128, E], fp32, tag="sc")
                nc.vector.tensor_tensor(
                    out=sc[:], in0=scores[:], in1=biasB[:], op=Alu.add)

                gs = rp.tile([128, N_GROUP], fp32, tag="gs")
                for g in range(N_GROUP):
                    g8 = rp.tile([128, 8], fp32, tag="g8")
                    nc.vector.max(g8[:], sc[:, g * 32:(g + 1) * 32])
                    nc.vector.reduce_sum(
                        gs[:, g:g + 1], g8[:, 0:2], axis=mybir.AxisListType.X)
                gs8 = rp.tile([128, 8], fp32, tag="gs8")
                nc.vector.max(gs8[:], gs[:])
                gmask = rp.tile([128, N_GROUP], fp32, tag="gmask")
                nc.vector.tensor_scalar(
                    out=gmask[:], in0=gs[:],
                    scalar1=gs8[:, TOPK_GROUP - 1:TOPK_GROUP],
                    scalar2=None, op0=Alu.is_ge)
                gpen = rp.tile([128, N_GROUP], fp32, tag="gpen")
                nc.vector.tensor_scalar(
                    out=gpen[:], in0=gmask[:], scalar1=1.0, scalar2=1e30,
                    op0=Alu.subtract, op1=Alu.mult)
                epen = rp.tile([128, E], fp32, tag="epen")
                nc.vector.tensor_copy(
                    epen[:].rearrange("p (g j) -> p g j", g=N_GROUP),
                    gpen[:].unsqueeze(2).broadcast_to([128, N_GROUP, 32]))
                masked = rp.tile([128, E], fp32, tag="masked")
                nc.vector.tensor_tensor(
                    out=masked[:], in0=sc[:], in1=epen[:], op=Alu.add)
                t8 = rp.tile([128, 8], fp32, tag="t8")
                nc.vector.max(t8[:], masked[:])
                sel = rp.tile([128, E], fp32, tag="sel")
                nc.vector.tensor_scalar(
                    out=sel[:], in0=masked[:],
                    scalar1=t8[:, TOP_K - 1:TOP_K],
                    scalar2=None, op0=Alu.is_ge)
                wsel = rp.tile([128, E], fp32, tag="epen", name="wsel")
                sw = rp.tile([128, 1], fp32, tag="sw")
                nc.vector.scalar_tensor_tensor(
                    out=wsel[:], in0=scores[:], scalar=1.0, in1=sel[:],
                    op0=Alu.mult, op1=Alu.mult, accum_out=sw[:])
                swp = rp.tile([128, 1], fp32, tag="swp")
                nc.vector.tensor_scalar(
                    out=swp[:], in0=sw[:], scalar1=1e-20, scalar2=None,
                    op0=Alu.add)
                rn = rp.tile([128, 1], fp32, tag="rn")
                nc.vector.reciprocal(rn[:], swp[:])
                comb = rp.tile([128, E], fp32, tag="scores", name="comb")
                nc.vector.tensor_scalar(
                    out=comb[:], in0=wsel[:], scalar1=rn[:], scalar2=SCALE,
                    op0=Alu.mult, op1=Alu.mult)
                ps_c = tp.tile([128, 2, T], fp32, tag="ps")
                nc.tensor.transpose(
                    ps_c[0:EL, 0, 0:128], comb[:, 0:EL], ident[:])
                nc.vector.tensor_copy(
                    combT[:, t_c * 128:(t_c + 1) * 128], ps_c[0:EL, 0, 0:128])

            # ---------- helpers: expert up-projection + activation ----------
            a13_t = {}

            def emit_up(e):
                ensure_up_w(e + ahead)
                ensure_wd_w(e + ahead - 2)
                hh = hp.tile([128, 2, IC, T], fp32, tag="hh", name=f"hh{e}")
                w = wup[e]
                for mi in range(IC):
                    for hc in range(HC):
                        nc.tensor.matmul(
                            hh[:, 0, mi, :],
                            w[:, 0, hc * I + mi * 128:hc * I + (mi + 1) * 128],
                            xTr[:, hc, :],
                            start=(mi == 0 and hc == 0), stop=(hc == HC - 1),
                            skip_group_check=True)
                for mi in range(IC):
                    for hc in range(HC):
                        nc.tensor.matmul(
                            hh[:, 1, mi, :],
                            w[:, 1, hc * I + mi * 128:hc * I + (mi + 1) * 128],
                            xTr[:, hc, :],
                            start=(mi == 0 and hc == 0), stop=(hc == HC - 1),
                            skip_group_check=True)
                s1 = s1p.tile([128, IC, T], fp16, tag="s1", name=f"s1_{e}")
                nc.scalar.activation(s1[:], hh[:, 0, :, :], Act.Silu)
                a13 = a13p.tile([128, IC, T], fp16, tag="a13", name=f"a13_{e}")
                nc.vector.tensor_tensor(
                    out=a13[:], in0=hh[:, 1, :, :], in1=s1[:], op=Alu.mult)
                a13_t[e] = a13

            # shared expert up-path (no routing dependency)
            hsu = hp.tile([IS, 2, IC, T], fp32, tag="hh")
            for hc in range(HC):
                nc.tensor.matmul(
                    hsu[:, 0, 0, :], swg_t[:, hc, :], xTr[:, hc, :],
                    start=(hc == 0), stop=(hc == HC - 1),
                    skip_group_check=True)
            for hc in range(HC):
                nc.tensor.matmul(
                    hsu[:, 1, 0, :], swu_t[:, hc, :], xTr[:, hc, :],
                    start=(hc == 0), stop=(hc == HC - 1),
                    skip_group_check=True)
            s_s1 = sp.tile([IS, T], fp16, tag="ss1")
            nc.scalar.activation(s_s1[:], hsu[:, 0, 0, :], Act.Silu)
            s_act = sp.tile([IS, T], fp16, tag="sact")
            nc.vector.tensor_tensor(
                out=s_act[:], in0=hsu[:, 1, 0, :], in1=s_s1[:], op=Alu.mult)

            # first experts' up-path keeps PE busy while routing DVE runs
            for e in range(pre_n):
                emit_up(e)

            # broadcast combT rows to all 128 partitions via PE:
            for j in range(EL // 2):
                cb_ps = tp.tile([128, 2, T], fp32, tag="ps")
                for h in range(2):
                    e = 2 * j + h
                    nc.tensor.matmul(
                        cb_ps[:, h, :],
                        onehotE[:, e:e + 1].broadcast_to([EL, 128]),
                        combT[:], start=True, stop=True,
                        skip_group_check=True)
                nc.scalar.copy(CB_all[:, 2 * j:2 * j + 2, :], cb_ps[:])

          # ---------- experts ----------
          with tc.tile_pool(name="ypsum", bufs=1, space="PSUM") as yp:
            y_ps = yp.tile([128, TC, H], fp32)   # Y[t, h] accumulator

            # shared expert down-projection first: only needs s_act, and
            # keeps it off the critical tail after the last expert
            for t_c in range(TC):
                for nh in range(2):
                    nc.tensor.matmul(
                        y_ps[:, t_c, nh * 512:(nh + 1) * 512],
                        s_act[:, t_c * 128:(t_c + 1) * 128],
                        swd_t[:, nh * 512:(nh + 1) * 512],
                        start=True, stop=False,
                        skip_group_check=True)

            for e in range(EL):
                if e >= pre_n:
                    emit_up(e)
                act_t = s1p.tile([128, IC, T], fp16, tag="act", name=f"act{e}")
                nc.vector.tensor_tensor(
                    out=act_t[:], in0=a13_t.pop(e)[:],
                    in1=CB_all[:, e, :].unsqueeze(1).broadcast_to([128, IC, T]),
                    op=Alu.mult)

                wdv = wdn[e][:].rearrange("p (c h) -> p c h", c=IC)
                for t_c in range(TC):
                    for nh in range(2):
                        for ic in range(IC):
                            nc.tensor.matmul(
                                y_ps[:, t_c, nh * 512:(nh + 1) * 512],
                                act_t[:, ic, t_c * 128:(t_c + 1) * 128],
                                wdv[:, ic, nh * 512:(nh + 1) * 512],
                                start=False,
                                stop=(e == EL - 1 and ic == IC - 1),
                                skip_group_check=True)

            # ---------- copy out (+ AllReduce in multi-core) ----------
            if single_core:
                for t_c in range(TC):
                    for half in range(2):
                        k = 2 * t_c + half
                        yo = s1p.tile([128, 512], fp32,
                                      tag=("act" if k % 2 == 0 else "s1"),
                                      name=f"yo{t_c}_{half}")
                        if k % 2 == 0:
                            nc.vector.tensor_copy(
                                yo[:], y_ps[:, t_c, half * 512:(half + 1) * 512])
                        else:
                            nc.scalar.copy(
                                yo[:], y_ps[:, t_c, half * 512:(half + 1) * 512])
                        nc.sync.dma_start(
                            Y.ap()[t_c * 128:(t_c + 1) * 128,
                                   half * 512:(half + 1) * 512], yo[:])
            else:
                in_b = dp.tile([T, H], fp32)
                out_b = dp.tile([T, H], fp32, addr_space="Shared")
                for t_c in range(TC):
                    for half in range(2):
                        k = 2 * t_c + half
                        yo = s1p.tile([128, 512], fp32,
                                      tag=("act" if k % 2 == 0 else "s1"),
                                      name=f"yo{t_c}_{half}")
                        if k % 2 == 0:
                            nc.vector.tensor_copy(
                                yo[:], y_ps[:, t_c, half * 512:(half + 1) * 512])
                        else:
                            nc.scalar.copy(
                                yo[:], y_ps[:, t_c, half * 512:(half + 1) * 512])
                        nc.sync.dma_start(
                            in_b[t_c * 128:(t_c + 1) * 128,
                                 half * 512:(half + 1) * 512], yo[:])
                nc.gpsimd.collective_compute(
                    "AllReduce", Alu.add,
                    replica_groups=[list(range(N_CORES))],
                    ins=[in_b.opt()], outs=[out_b.opt()])
                nc.sync.dma_start(Y.ap(), out_b[:])

    nc.finalize()
    return nc


def _get_nc():
    if "nc" not in _NC_CACHE:
        _NC_CACHE["nc"] = build_nc()
    return _NC_CACHE["nc"]


def _sw(a):
    """[X, HC*128] -> [128, HC, X]-style partition-major swizzle."""
    n, h = a.shape
    return np.ascontiguousarray(a.reshape(n, HC, 128).transpose(2, 1, 0))


def make_in_maps(inputs):
    x = np.asarray(inputs["hidden_states"], dtype=np.float32).reshape(T, H)
    gate_w = np.asarray(inputs["gate_w"], dtype=np.float32)
    e_bias = np.asarray(inputs["e_bias"], dtype=np.float32)
    w_gate = np.asarray(inputs["w_gate"], dtype=np.float32)
    w_up = np.asarray(inputs["w_up"], dtype=np.float32)
    w_down = np.asarray(inputs["w_down"], dtype=np.float32)
    sw_gate = np.asarray(inputs["sw_gate"], dtype=np.float32)
    sw_up = np.asarray(inputs["sw_up"], dtype=np.float32)
    sw_down = np.asarray(inputs["sw_down"], dtype=np.float32)

    xt = _sw(x)  # [128, HC, T]
    in_maps = []
    for c in range(N_CORES):
        order = [(c + k) % N_GROUP for k in range(N_GROUP)]
        perm = np.concatenate([np.arange(g * EL, (g + 1) * EL) for g in order])
        sl = slice(c * EL, (c + 1) * EL)
        # per-expert packs, partition-major
        bu = np.empty((EL, 128, 2, 2048), np.float32)
        bu[:, :, 0, :] = w_gate[sl].reshape(EL, HC, 128, I).transpose(
            0, 2, 1, 3).reshape(EL, 128, HC * I)
        bu[:, :, 1, :] = w_up[sl].reshape(EL, HC, 128, I).transpose(
            0, 2, 1, 3).reshape(EL, 128, HC * I)
        bd = np.ascontiguousarray(
            w_down[sl].reshape(EL, IC, 128, H).transpose(
                0, 2, 1, 3).reshape(EL, 128, IC * H))
        in_maps.append({
            "xt": xt,
            "gwt": _sw(np.ascontiguousarray(gate_w[perm])),
            "ebp": np.ascontiguousarray(e_bias[perm]),
            "wbu": bu.astype(np.float16),
            "wbd": bd.astype(np.float16),
            "swgt": _sw(np.ascontiguousarray(
                sw_gate[:, c * IS:(c + 1) * IS].T)).astype(np.float16),
            "swut": _sw(np.ascontiguousarray(
                sw_up[:, c * IS:(c + 1) * IS].T)).astype(np.float16),
            "swd": np.ascontiguousarray(
                sw_down[c * IS:(c + 1) * IS, :]).astype(np.float16),
        })
    return in_maps


def kernel(**inputs) -> np.ndarray:
    nc = _get_nc()
    in_maps = make_in_maps(inputs)
    res = run_bass_kernel_spmd(nc, in_maps, core_ids=list(range(N_CORES)))
    y = res.results[0]["y"]
    return np.asarray(y, dtype=np.float32).reshape(1, 1, T, H)


if __name__ == "__main__":
    rng = np.random.default_rng(0)
    demo = {
        "hidden_states": rng.standard_normal((1, 1, T, H)).astype(np.float32),
        "gate_w": (rng.standard_normal((E, H)) / np.sqrt(H)).astype(np.float32),
        "e_bias": (rng.standard_normal(E) * 0.1).astype(np.float32),
        "w_gate": (rng.standard_normal((E, H, I)) / np.sqrt(H)).astype(np.float32),
        "w_up": (rng.standard_normal((E, H, I)) / np.sqrt(H)).astype(np.float32),
        "w_down": (rng.standard_normal((E, I, H)) / np.sqrt(I)).astype(np.float32),
        "sw_gate": (rng.standard_normal((H, I)) / np.sqrt(H)).astype(np.float32),
        "sw_up": (rng.standard_normal((H, I)) / np.sqrt(H)).astype(np.float32),
        "sw_down": (rng.standard_normal((I, H)) / np.sqrt(I)).astype(np.float32),
    }
    out = kernel(**demo)
    print("kernel output:", out.shape, out.dtype, np.abs(out).max())
